# revision 5
# baseline (speedup 1.0000x reference)
"""ChebConv (K=6) Trainium2 kernel — node-sharded bf16 version.

Cores = NS node shards x (8/NS) batch groups. Each core owns NBS=ceil(157/NS)
dst blocks for BL=NS batches (C = BL*64 features per row). Per Chebyshev step:
COO gather of src rows (bf16, C*2 bytes each, 4 SWDGE queues) + selection-matrix
matmuls accumulate 128-row dst blocks in PSUM; an AllGather over the shard
group rebuilds the full T_k for the next step's gathers. The trailing fc runs
on own-shard slices only (batch-pair block-diagonal weight trick halves the
transpose count).

DRAM layouts are partition-major permuted per shard (AG row of node n:
shard*128*NBS + (n%128)*NBS + local_block) so staging DMAs stay contiguous.
"""

import numpy as np
import ml_dtypes
import concourse.bacc as bacc
import concourse.mybir as mybir
from concourse.tile import TileContext
from concourse.bass_utils import run_bass_kernel_spmd

P = 128
F = 64
OUT = 64
K_CHEB = 6
N_CORES = 8
NS = 4  # node shards (4 => 2 batch groups of 4 batches)

BF16 = ml_dtypes.bfloat16


def _prep_graph(rows, cols, vals, n_blocks157, nbs):
    """Per-shard edge slots, sorted by dst row, padded to a common NCPB."""
    order = np.argsort(rows, kind="stable")
    rows_s = rows[order].astype(np.int64)
    cols_s = cols[order].astype(np.int64)
    vals_s = vals[order].astype(np.float32)
    blk = rows_s // P
    counts = np.bincount(blk, minlength=NS * nbs)
    ncpb = int(-(-counts.max() // P))
    spb = ncpb * P  # slots per block
    es = nbs * spb  # slots per core
    # AG row for source node c
    cblk = cols_s // P
    ag_row = (cblk // nbs) * (P * nbs) + (cols_s % P) * nbs + (cblk % nbs)

    starts = np.zeros(NS * nbs, np.int64)
    starts[1:] = np.cumsum(counts)[:-1]
    idx_all, vals_all, rowloc_all = [], [], []
    for s in range(NS):
        # pad slots gather row 0 with val 0 (keeps num_idxs_reg == ge exact,
        # required because the SPMD NEFF bakes one immediate for all cores)
        idx_p = np.zeros(es, np.int64)
        vals_p = np.zeros(es, np.float32)
        rowloc_p = np.zeros(es, np.float32)
        lo = starts[s * nbs]
        hi = lo + counts[s * nbs:(s + 1) * nbs].sum()
        sel = slice(lo, hi)
        pos = (np.arange(lo, hi) - starts[blk[sel]]) + (blk[sel] - s * nbs) * spb
        idx_p[pos] = ag_row[sel]
        vals_p[pos] = 2.0 * vals_s[sel]
        rowloc_p[pos] = (rows_s[sel] % P).astype(np.float32)
        idx_all.append(np.tile(idx_p.astype(np.int16).reshape(-1, 16).T, (8, 1)).copy())
        nchunk = es // P
        vals_all.append(np.ascontiguousarray(vals_p.reshape(nchunk, P).T))
        rowloc_all.append(np.ascontiguousarray(rowloc_p.reshape(nchunk, P).T))
    return idx_all, vals_all, rowloc_all, ncpb


def build_kernel(nbs, ncpb, do_gather=True, do_compute=True, do_fc=True,
                 do_cc=True):
    bl = NS  # batches per core
    C = bl * F
    es = nbs * ncpb * P
    nchunk = es // P
    ge = ncpb * P
    nag = NS * P * nbs
    dt = mybir.dt.bfloat16
    f32 = mybir.dt.float32
    npairs = bl // 2

    nc = bacc.Bacc(None, target_bir_lowering=False, num_swdge_queues=4)
    x_ag_in = nc.dram_tensor("x_ag", [nag, C], dt, kind="ExternalInput")
    x_own_in = nc.dram_tensor("x_own", [P, nbs * C], dt, kind="ExternalInput")
    idx_in = nc.dram_tensor("idx_in", [P, es // 16], mybir.dt.int16,
                            kind="ExternalInput")
    vals_in = nc.dram_tensor("vals_in", [P, nchunk], f32, kind="ExternalInput")
    rowloc_in = nc.dram_tensor("rowloc_in", [P, nchunk], f32,
                               kind="ExternalInput")
    iota_in = nc.dram_tensor("iota_in", [P, P], f32, kind="ExternalInput")
    ident_in = nc.dram_tensor("ident_in", [P, P], dt, kind="ExternalInput")
    wt_in = nc.dram_tensor("wt_in", [P, K_CHEB * P], dt, kind="ExternalInput")
    b_in = nc.dram_tensor("b_in", [1, C], dt, kind="ExternalInput")
    out_dram = nc.dram_tensor("out", [P, nbs * C], f32, kind="ExternalOutput")

    groups = [[g * NS + s for s in range(NS)] for g in range(N_CORES // NS)]

    with TileContext(nc) as tc:
        with (
            tc.tile_pool(name="const", bufs=1) as cpool,
            tc.tile_pool(name="dram", bufs=1, space="DRAM") as dpool,
            tc.tile_pool(name="gbuf", bufs=4) as gpool,
            tc.tile_pool(name="spool", bufs=4) as spool,
            tc.tile_pool(name="fcin", bufs=2 * K_CHEB) as fcpool,
            tc.tile_pool(name="stg", bufs=2) as stgpool,
            tc.tile_pool(name="ttp", bufs=3) as ttpool,
            tc.tile_pool(name="psA", bufs=4, space="PSUM") as psA,
            tc.tile_pool(name="psT", bufs=2, space="PSUM") as psT,
            tc.tile_pool(name="psF", bufs=2, space="PSUM") as psF,
        ):
            iota_sb = cpool.tile([P, P], f32, tag="iota")
            nc.sync.dma_start(iota_sb[:], iota_in[:])
            ident_sb = cpool.tile([P, P], dt, tag="ident")
            nc.sync.dma_start(ident_sb[:], ident_in[:])
            vals_sb = cpool.tile([P, nchunk], f32, tag="vals")
            nc.sync.dma_start(vals_sb[:], vals_in[:])
            rowloc_sb = cpool.tile([P, nchunk], f32, tag="rowloc")
            nc.sync.dma_start(rowloc_sb[:], rowloc_in[:])
            wt_sb = cpool.tile([P, K_CHEB * P], dt, tag="wt")
            nc.sync.dma_start(wt_sb[:], wt_in[:])
            b_sb = cpool.tile([1, C], dt, tag="bias")
            nc.sync.dma_start(b_sb[:], b_in[:])
            ones_sb = cpool.tile([1, P], dt, tag="ones")
            nc.vector.memset(ones_sb[:], 1.0)
            idx_sb = cpool.tile([P, es // 16], mybir.dt.int16, tag="idx")
            nc.sync.dma_start(idx_sb[:], idx_in[:])

            tb = [
                cpool.tile([P, nbs, C], dt, tag="tb0", name="tb0"),
                cpool.tile([P, nbs, C], dt, tag="tb1", name="tb1"),
            ]
            nc.sync.dma_start(
                tb[0][:, :, :].rearrange("p b f -> p (b f)"), x_own_in[:, :]
            )

            for _gi in range(4):
                g_init = gpool.tile([P, ncpb, C], dt, tag="g", name=f"gi{_gi}")
                nc.gpsimd.memset(g_init[:], 0.0)

            t_sl = [None] + [
                dpool.tile([P, nbs * C], dt, tag=f"tsl{k}", name=f"tsl{k}")
                for k in range(1, K_CHEB)
            ]
            t_ag = [x_ag_in] + [
                dpool.tile([nag, C], dt, tag=f"tag{k}", name=f"tag{k}")
                for k in range(1, K_CHEB - 1)
            ]

            # ---- Chebyshev recurrence: 5 SPMM passes
            for k in range(1, K_CHEB):
                src = t_ag[k - 1]
                tbc = tb[k % 2]
                for j in range(nbs):
                    g_tile = gpool.tile([P, ncpb, C], dt, tag="g")
                    if do_gather:
                        nc.gpsimd.dma_gather(
                            g_tile[:],
                            src[:, :],
                            idx_sb[:, j * (ge // 16):(j + 1) * (ge // 16)],
                            ge,
                            ge,
                            C,
                            single_packet=False,
                            queue_num=j % 4,
                        )
                    psum = psA.tile([P, C], f32)
                    if not do_compute:
                        continue
                    for c in range(ncpb):
                        gc = j * ncpb + c
                        s_tile = spool.tile([P, P], dt, tag="s")
                        nc.vector.tensor_scalar(
                            out=s_tile[:],
                            in0=iota_sb[:],
                            scalar1=rowloc_sb[:, gc:gc + 1],
                            scalar2=vals_sb[:, gc:gc + 1],
                            op0=mybir.AluOpType.is_equal,
                            op1=mybir.AluOpType.mult,
                        )
                        nc.tensor.matmul(
                            psum[:],
                            s_tile[:],
                            g_tile[:, c, :],
                            start=(c == 0),
                            stop=(c == ncpb - 1),
                        )
                    if k == 1:
                        nc.vector.tensor_scalar(
                            out=tbc[:, j, :], in0=psum[:],
                            scalar1=0.5, scalar2=None,
                            op0=mybir.AluOpType.mult,
                        )
                    else:
                        nc.vector.tensor_tensor(
                            out=tbc[:, j, :], in0=psum[:],
                            in1=tbc[:, j, :], op=mybir.AluOpType.subtract,
                        )
                nc.sync.dma_start(
                    t_sl[k][:, :], tbc[:, :, :].rearrange("p b f -> p (b f)")
                )
                if k < K_CHEB - 1 and do_cc:
                    nc.gpsimd.collective_compute(
                        "AllGather",
                        mybir.AluOpType.bypass,
                        replica_groups=groups,
                        ins=[t_sl[k].opt()],
                        outs=[t_ag[k].opt()],
                    )

            # ---- fc on own-shard slices (batch-pair blkdiag weights)
            fc_src = [x_own_in] + t_sl[1:]
            wbf = 8
            nbatch = (-(-nbs // wbf)) if do_fc else 0
            for bt in range(nbatch):
                b0 = bt * wbf
                nblk = min(wbf, nbs - b0)
                fc_t = []
                for k in range(K_CHEB):
                    t_t = fcpool.tile([P, wbf, C], dt, tag="fcin")
                    nc.sync.dma_start(
                        t_t[:, :nblk, :].rearrange("p b f -> p (b f)"),
                        fc_src[k][:, b0 * C:(b0 + nblk) * C],
                    )
                    fc_t.append(t_t)
                ostg = stgpool.tile([P, wbf, C], f32, tag="ostg")
                for j in range(nblk):
                    fc_psum = psF.tile([P, C], f32)
                    for h in range(npairs):
                        for k in range(K_CHEB):
                            tps = psT.tile([P, P], dt)
                            nc.tensor.transpose(
                                out=tps[:],
                                in_=fc_t[k][:, j, h * P:(h + 1) * P],
                                identity=ident_sb[:],
                            )
                            tt_sb = ttpool.tile([P, P], dt, tag="tt")
                            nc.scalar.copy(out=tt_sb[:], in_=tps[:])
                            nc.tensor.matmul(
                                fc_psum[:, h * P:(h + 1) * P],
                                tt_sb[:],
                                wt_sb[:, k * P:(k + 1) * P],
                                start=(k == 0),
                                stop=False,
                            )
                        nc.tensor.matmul(
                            fc_psum[:, h * P:(h + 1) * P],
                            ones_sb[:],
                            b_sb[:, h * P:(h + 1) * P],
                            start=False,
                            stop=True,
                        )
                    nc.vector.tensor_copy(out=ostg[:, j, :], in_=fc_psum[:])
                nc.sync.dma_start(
                    out_dram[:, b0 * C:(b0 + nblk) * C],
                    ostg[:, :nblk, :].rearrange("p b f -> p (b f)"),
                )
    nc.finalize()
    return nc


def _host_prep(x, lap_rows, lap_cols, lap_vals, W, b):
    B, N, Fin = x.shape
    bl = NS
    nbg = N_CORES // NS
    C = bl * Fin
    nb157 = -(-N // P)
    nbs = -(-nb157 // NS)
    n_padded = NS * nbs * P

    idx_all, vals_all, rowloc_all, ncpb = _prep_graph(
        lap_rows, lap_cols, lap_vals, nb157, nbs
    )

    iota = np.tile(np.arange(P, dtype=np.float32), (P, 1))
    ident = np.eye(P, dtype=BF16)
    # wt_blk[f + 64e, k*128 + o + 64e] = W[o, k*F + f]
    wt = np.zeros((P, K_CHEB * P), np.float32)
    Wk = W.reshape(OUT, K_CHEB, Fin)  # [o, k, f]
    for k in range(K_CHEB):
        blkd = np.zeros((P, P), np.float32)
        blkd[:Fin, :OUT] = Wk[:, k, :].T
        blkd[Fin:, OUT:] = Wk[:, k, :].T
        wt[:, k * P:(k + 1) * P] = blkd
    wt = wt.astype(BF16)
    b_row = np.tile(b, bl).reshape(1, C).astype(BF16)

    # x in AG layout per batch group: [NS, P, nbs, C]
    xp = np.zeros((B, n_padded, Fin), np.float32)
    xp[:, :N] = x
    x_ag_groups = []
    for g in range(nbg):
        xg = xp[g * bl:(g + 1) * bl]  # [bl, n_padded, F]
        arr = xg.reshape(bl, NS, nbs, P, Fin).transpose(1, 3, 2, 0, 4)
        x_ag_groups.append(
            np.ascontiguousarray(arr.reshape(NS * P * nbs, C)).astype(BF16)
        )
    return (idx_all, vals_all, rowloc_all, ncpb, nbs, iota, ident, wt, b_row,
            x_ag_groups)


def kernel(x, lap_rows, lap_cols, lap_vals, W, b):
    x = np.asarray(x, dtype=np.float32)
    lap_rows = np.asarray(lap_rows, dtype=np.int32)
    lap_cols = np.asarray(lap_cols, dtype=np.int32)
    lap_vals = np.asarray(lap_vals, dtype=np.float32)
    W = np.asarray(W, dtype=np.float32)
    b = np.asarray(b, dtype=np.float32)
    B, N, Fin = x.shape
    bl = NS
    C = bl * Fin

    (idx_all, vals_all, rowloc_all, ncpb, nbs, iota, ident, wt, b_row,
     x_ag_groups) = _host_prep(x, lap_rows, lap_cols, lap_vals, W, b)

    nc = build_kernel(nbs, ncpb)
    in_maps = []
    for core in range(N_CORES):
        g, s = core // NS, core % NS
        x_ag = x_ag_groups[g]
        x_own = np.ascontiguousarray(
            x_ag[s * P * nbs:(s + 1) * P * nbs].reshape(P, nbs * C)
        )
        in_maps.append({
            "x_ag": x_ag,
            "x_own": x_own,
            "idx_in": idx_all[s],
            "vals_in": vals_all[s],
            "rowloc_in": rowloc_all[s],
            "iota_in": iota,
            "ident_in": ident,
            "wt_in": wt,
            "b_in": b_row,
        })
    res = run_bass_kernel_spmd(nc, in_maps, core_ids=list(range(N_CORES)))

    out = np.zeros((B, N, OUT), np.float32)
    for core in range(N_CORES):
        g, s = core // NS, core % NS
        o = res.results[core]["out"].reshape(P, nbs, bl, OUT)
        for j in range(nbs):
            n0 = (s * nbs + j) * P
            if n0 >= N:
                break
            n1 = min(n0 + P, N)
            out[g * bl:(g + 1) * bl, n0:n1] = (
                o[:n1 - n0, j].transpose(1, 0, 2)
            )
    return out


# revision 6
# speedup vs baseline: 29.1088x; 29.1088x over previous
"""ChebConv (K=6) Trainium2 kernel — node-sharded bf16 version.

Cores = NS node shards x (8/NS) batch groups. Each core owns NBS=ceil(157/NS)
dst blocks for BL=NS batches (C = BL*64 features per row). Per Chebyshev step:
COO gather of src rows (bf16, C*2 bytes each, 4 SWDGE queues) + selection-matrix
matmuls accumulate 128-row dst blocks in PSUM; an AllGather over the shard
group rebuilds the full T_k for the next step's gathers. The trailing fc runs
on own-shard slices only (batch-pair block-diagonal weight trick halves the
transpose count).

DRAM layouts are partition-major permuted per shard (AG row of node n:
shard*128*NBS + (n%128)*NBS + local_block) so staging DMAs stay contiguous.
"""

import numpy as np
import ml_dtypes
import concourse.bacc as bacc
import concourse.mybir as mybir
from concourse.tile import TileContext
from concourse.bass_utils import run_bass_kernel_spmd

P = 128
F = 64
OUT = 64
K_CHEB = 6
N_CORES = 8
NS = 4  # node shards (4 => 2 batch groups of 4 batches)

BF16 = ml_dtypes.bfloat16


def _prep_graph(rows, cols, vals, n_blocks157, nbs):
    """Per-shard edge slots, sorted by dst row, padded to a common NCPB."""
    order = np.argsort(rows, kind="stable")
    rows_s = rows[order].astype(np.int64)
    cols_s = cols[order].astype(np.int64)
    vals_s = vals[order].astype(np.float32)
    blk = rows_s // P
    counts = np.bincount(blk, minlength=NS * nbs)
    ncpb = int(-(-counts.max() // P))
    spb = ncpb * P  # slots per block
    es = nbs * spb  # slots per core
    # AG row for source node c (2-chunk layout: chunk q = local_block//hb)
    hb = nbs // 2
    cblk = cols_s // P
    j_loc = cblk % nbs
    ag_row = ((j_loc // hb) * (NS * P * hb) + (cblk // nbs) * (P * hb)
              + (cols_s % P) * hb + (j_loc % hb))

    starts = np.zeros(NS * nbs, np.int64)
    starts[1:] = np.cumsum(counts)[:-1]
    idx_all, vals_all, rowloc_all = [], [], []
    for s in range(NS):
        # pad slots gather row 0 with val 0 (keeps num_idxs_reg == ge exact,
        # required because the SPMD NEFF bakes one immediate for all cores)
        idx_p = np.zeros(es, np.int64)
        vals_p = np.zeros(es, np.float32)
        rowloc_p = np.zeros(es, np.float32)
        lo = starts[s * nbs]
        hi = lo + counts[s * nbs:(s + 1) * nbs].sum()
        sel = slice(lo, hi)
        pos = (np.arange(lo, hi) - starts[blk[sel]]) + (blk[sel] - s * nbs) * spb
        idx_p[pos] = ag_row[sel]
        vals_p[pos] = 2.0 * vals_s[sel]
        rowloc_p[pos] = (rows_s[sel] % P).astype(np.float32)
        idx_all.append(np.tile(idx_p.astype(np.int16).reshape(-1, 16).T, (8, 1)).copy())
        nchunk = es // P
        vals_all.append(np.ascontiguousarray(vals_p.reshape(nchunk, P).T))
        rowloc_all.append(np.ascontiguousarray(rowloc_p.reshape(nchunk, P).T))
    return idx_all, vals_all, rowloc_all, ncpb


def build_kernel(nbs, ncpb, do_gather=True, do_compute=True, do_fc=True,
                 do_cc=True):
    bl = NS  # batches per core
    C = bl * F
    es = nbs * ncpb * P
    nchunk = es // P
    ge = ncpb * P
    nag = NS * P * nbs
    dt = mybir.dt.bfloat16
    f32 = mybir.dt.float32
    npairs = bl // 2

    nc = bacc.Bacc(None, target_bir_lowering=False, num_swdge_queues=4)
    x_ag_in = nc.dram_tensor("x_ag", [nag, C], dt, kind="ExternalInput")
    x_own_in = nc.dram_tensor("x_own", [P, nbs * C], dt, kind="ExternalInput")
    idx_in = nc.dram_tensor("idx_in", [P, es // 16], mybir.dt.int16,
                            kind="ExternalInput")
    vals_in = nc.dram_tensor("vals_in", [P, nchunk], f32, kind="ExternalInput")
    rowloc_in = nc.dram_tensor("rowloc_in", [P, nchunk], f32,
                               kind="ExternalInput")
    iota_in = nc.dram_tensor("iota_in", [P, P], f32, kind="ExternalInput")
    ident_in = nc.dram_tensor("ident_in", [P, P], dt, kind="ExternalInput")
    wt_in = nc.dram_tensor("wt_in", [P, K_CHEB * P], dt, kind="ExternalInput")
    b_in = nc.dram_tensor("b_in", [1, C], dt, kind="ExternalInput")
    out_dram = nc.dram_tensor("out", [P, nbs * C], f32, kind="ExternalOutput")

    groups = [[g * NS + s for s in range(NS)] for g in range(N_CORES // NS)]

    with TileContext(nc) as tc:
        with (
            tc.tile_pool(name="const", bufs=1) as cpool,
            tc.tile_pool(name="dram", bufs=1, space="DRAM") as dpool,
            tc.tile_pool(name="gbuf", bufs=4) as gpool,
            tc.tile_pool(name="spool", bufs=4) as spool,
            tc.tile_pool(name="fcin", bufs=K_CHEB) as fcpool,
            tc.tile_pool(name="stg", bufs=2) as stgpool,
            tc.tile_pool(name="ttp", bufs=3) as ttpool,
            tc.tile_pool(name="psA", bufs=4, space="PSUM") as psA,
            tc.tile_pool(name="psT", bufs=2, space="PSUM") as psT,
            tc.tile_pool(name="psF", bufs=2, space="PSUM") as psF,
        ):
            iota_sb = cpool.tile([P, P], f32, tag="iota")
            nc.sync.dma_start(iota_sb[:], iota_in[:])
            ident_sb = cpool.tile([P, P], dt, tag="ident")
            nc.sync.dma_start(ident_sb[:], ident_in[:])
            vals_sb = cpool.tile([P, nchunk], f32, tag="vals")
            nc.sync.dma_start(vals_sb[:], vals_in[:])
            rowloc_sb = cpool.tile([P, nchunk], f32, tag="rowloc")
            nc.sync.dma_start(rowloc_sb[:], rowloc_in[:])
            wt_sb = cpool.tile([P, K_CHEB * P], dt, tag="wt")
            nc.sync.dma_start(wt_sb[:], wt_in[:])
            b_sb = cpool.tile([1, C], dt, tag="bias")
            nc.sync.dma_start(b_sb[:], b_in[:])
            ones_sb = cpool.tile([1, P], dt, tag="ones")
            nc.vector.memset(ones_sb[:], 1.0)
            idx_sb = cpool.tile([P, es // 16], mybir.dt.int16, tag="idx")
            nc.sync.dma_start(idx_sb[:], idx_in[:])

            tb = [
                cpool.tile([P, nbs, C], dt, tag="tb0", name="tb0"),
                cpool.tile([P, nbs, C], dt, tag="tb1", name="tb1"),
            ]
            nc.sync.dma_start(
                tb[0][:, :, :].rearrange("p b f -> p (b f)"), x_own_in[:, :]
            )

            for _gi in range(4):
                g_init = gpool.tile([P, ncpb, C], dt, tag="g", name=f"gi{_gi}")
                nc.gpsimd.memset(g_init[:], 0.0)

            hb = nbs // 2
            t_sl = [None] + [
                [dpool.tile([P, hb * C], dt, tag=f"tsl{k}{q}",
                            name=f"tsl{k}{q}") for q in range(2)]
                for k in range(1, K_CHEB)
            ]
            t_ag = [x_ag_in] + [
                dpool.tile([nag, C], dt, tag=f"tag{k}", name=f"tag{k}")
                for k in range(1, K_CHEB - 1)
            ]

            # ---- Chebyshev recurrence: 5 SPMM passes
            for k in range(1, K_CHEB):
                src = t_ag[k - 1]
                tbc = tb[k % 2]
                for j in range(nbs):
                    g_tile = gpool.tile([P, ncpb, C], dt, tag="g")
                    if do_gather:
                        nc.gpsimd.dma_gather(
                            g_tile[:],
                            src[:, :],
                            idx_sb[:, j * (ge // 16):(j + 1) * (ge // 16)],
                            ge,
                            ge,
                            C,
                            single_packet=False,
                            queue_num=j % 4,
                        )
                    psum = psA.tile([P, C], f32)
                    if not do_compute:
                        continue
                    for c in range(ncpb):
                        gc = j * ncpb + c
                        s_tile = spool.tile([P, P], dt, tag="s")
                        nc.vector.tensor_scalar(
                            out=s_tile[:],
                            in0=iota_sb[:],
                            scalar1=rowloc_sb[:, gc:gc + 1],
                            scalar2=vals_sb[:, gc:gc + 1],
                            op0=mybir.AluOpType.is_equal,
                            op1=mybir.AluOpType.mult,
                        )
                        nc.tensor.matmul(
                            psum[:],
                            s_tile[:],
                            g_tile[:, c, :],
                            start=(c == 0),
                            stop=(c == ncpb - 1),
                        )
                    if k == 1:
                        nc.vector.tensor_scalar(
                            out=tbc[:, j, :], in0=psum[:],
                            scalar1=0.5, scalar2=None,
                            op0=mybir.AluOpType.mult,
                        )
                    else:
                        nc.vector.tensor_tensor(
                            out=tbc[:, j, :], in0=psum[:],
                            in1=tbc[:, j, :], op=mybir.AluOpType.subtract,
                        )
                    if do_compute and (j == hb - 1 or j == nbs - 1):
                        q = j // hb
                        nc.sync.dma_start(
                            t_sl[k][q][:, :],
                            tbc[:, q * hb:(q + 1) * hb, :].rearrange(
                                "p b f -> p (b f)"),
                        )
                        if k < K_CHEB - 1 and do_cc:
                            nc.gpsimd.collective_compute(
                                "AllGather",
                                mybir.AluOpType.bypass,
                                replica_groups=groups,
                                ins=[t_sl[k][q][:, :]],
                                outs=[t_ag[k][q * NS * P * hb:
                                              (q + 1) * NS * P * hb, :]],
                            )


            # ---- fc on own-shard slices (batch-pair blkdiag weights)
            wbf = 10
            nbatch = (-(-nbs // wbf)) if do_fc else 0
            for bt in range(nbatch):
                b0 = bt * wbf
                nblk = min(wbf, nbs - b0)
                fc_t = []
                q, off = b0 // hb, b0 % hb
                for k in range(K_CHEB):
                    t_t = fcpool.tile([P, wbf, C], dt, tag="fcin")
                    src_ap = (x_own_in[:, b0 * C:(b0 + nblk) * C] if k == 0
                              else t_sl[k][q][:, off * C:(off + nblk) * C])
                    nc.sync.dma_start(
                        t_t[:, :nblk, :].rearrange("p b f -> p (b f)"), src_ap
                    )
                    fc_t.append(t_t)
                ostg = stgpool.tile([P, wbf, C], f32, tag="ostg")
                for j in range(nblk):
                    fc_psum = psF.tile([P, C], f32)
                    for h in range(npairs):
                        for k in range(K_CHEB):
                            tps = psT.tile([P, P], dt)
                            nc.tensor.transpose(
                                out=tps[:],
                                in_=fc_t[k][:, j, h * P:(h + 1) * P],
                                identity=ident_sb[:],
                            )
                            tt_sb = ttpool.tile([P, P], dt, tag="tt")
                            nc.scalar.copy(out=tt_sb[:], in_=tps[:])
                            nc.tensor.matmul(
                                fc_psum[:, h * P:(h + 1) * P],
                                tt_sb[:],
                                wt_sb[:, k * P:(k + 1) * P],
                                start=(k == 0),
                                stop=False,
                            )
                        nc.tensor.matmul(
                            fc_psum[:, h * P:(h + 1) * P],
                            ones_sb[:],
                            b_sb[:, h * P:(h + 1) * P],
                            start=False,
                            stop=True,
                        )
                    nc.vector.tensor_copy(out=ostg[:, j, :], in_=fc_psum[:])
                nc.sync.dma_start(
                    out_dram[:, b0 * C:(b0 + nblk) * C],
                    ostg[:, :nblk, :].rearrange("p b f -> p (b f)"),
                )
    nc.finalize()
    return nc


def _host_prep(x, lap_rows, lap_cols, lap_vals, W, b):
    B, N, Fin = x.shape
    bl = NS
    nbg = N_CORES // NS
    C = bl * Fin
    nb157 = -(-N // P)
    nbs = -(-nb157 // NS)
    n_padded = NS * nbs * P

    idx_all, vals_all, rowloc_all, ncpb = _prep_graph(
        lap_rows, lap_cols, lap_vals, nb157, nbs
    )

    iota = np.tile(np.arange(P, dtype=np.float32), (P, 1))
    ident = np.eye(P, dtype=BF16)
    # wt_blk[f + 64e, k*128 + o + 64e] = W[o, k*F + f]
    wt = np.zeros((P, K_CHEB * P), np.float32)
    Wk = W.reshape(OUT, K_CHEB, Fin)  # [o, k, f]
    for k in range(K_CHEB):
        blkd = np.zeros((P, P), np.float32)
        blkd[:Fin, :OUT] = Wk[:, k, :].T
        blkd[Fin:, OUT:] = Wk[:, k, :].T
        wt[:, k * P:(k + 1) * P] = blkd
    wt = wt.astype(BF16)
    b_row = np.tile(b, bl).reshape(1, C).astype(BF16)

    # x in AG layout per batch group: [NS, P, nbs, C]
    xp = np.zeros((B, n_padded, Fin), np.float32)
    xp[:, :N] = x
    hb = nbs // 2
    x_ag_groups, x_own_all = [], []
    for g in range(nbg):
        xg = xp[g * bl:(g + 1) * bl]  # [bl, n_padded, F]
        arr = xg.reshape(bl, NS, 2, hb, P, Fin).transpose(2, 1, 4, 3, 0, 5)
        x_ag_groups.append(
            np.ascontiguousarray(arr.reshape(2 * NS * P * hb, C)).astype(BF16)
        )
        arr5 = xg.reshape(bl, NS, nbs, P, Fin)
        x_own_all.append([
            np.ascontiguousarray(
                arr5[:, s].transpose(2, 1, 0, 3).reshape(P, nbs * C)
            ).astype(BF16) for s in range(NS)
        ])
    return (idx_all, vals_all, rowloc_all, ncpb, nbs, iota, ident, wt, b_row,
            x_ag_groups, x_own_all)


def kernel(x, lap_rows, lap_cols, lap_vals, W, b):
    x = np.asarray(x, dtype=np.float32)
    lap_rows = np.asarray(lap_rows, dtype=np.int32)
    lap_cols = np.asarray(lap_cols, dtype=np.int32)
    lap_vals = np.asarray(lap_vals, dtype=np.float32)
    W = np.asarray(W, dtype=np.float32)
    b = np.asarray(b, dtype=np.float32)
    B, N, Fin = x.shape
    bl = NS
    C = bl * Fin

    (idx_all, vals_all, rowloc_all, ncpb, nbs, iota, ident, wt, b_row,
     x_ag_groups, x_own_all) = _host_prep(x, lap_rows, lap_cols, lap_vals, W, b)

    nc = build_kernel(nbs, ncpb)
    in_maps = []
    for core in range(N_CORES):
        g, s = core // NS, core % NS
        x_ag = x_ag_groups[g]
        x_own = x_own_all[g][s]
        in_maps.append({
            "x_ag": x_ag,
            "x_own": x_own,
            "idx_in": idx_all[s],
            "vals_in": vals_all[s],
            "rowloc_in": rowloc_all[s],
            "iota_in": iota,
            "ident_in": ident,
            "wt_in": wt,
            "b_in": b_row,
        })
    res = run_bass_kernel_spmd(nc, in_maps, core_ids=list(range(N_CORES)))

    out = np.zeros((B, N, OUT), np.float32)
    for core in range(N_CORES):
        g, s = core // NS, core % NS
        o = res.results[core]["out"].reshape(P, nbs, bl, OUT)
        for j in range(nbs):
            n0 = (s * nbs + j) * P
            if n0 >= N:
                break
            n1 = min(n0 + P, N)
            out[g * bl:(g + 1) * bl, n0:n1] = (
                o[:n1 - n0, j].transpose(1, 0, 2)
            )
    return out
